# revision 10
# baseline (speedup 1.0000x reference)
"""Trainium2 Bass kernel for the DirectEncoding GNN-message-passing model.

Problem (hardcoded):
  x [8192, 1024] f32; kernels [6, 64, 32, 8]; biases [6, 64, 8];
  idx [6, 64, 32] int32.  Per level l: gather 64 groups x 32 prior node
  columns from the growing activation buffer, per-group [32x8] matmul +
  bias + tanh, append the 512 new nodes.  Output = level-5 nodes
  [8192, 512] f32.

Strategy: pure data parallel over 8 NeuronCores (1024 batch rows each).
All indices are compile-time constants, so the irregular gather is baked
into the program:

  - node-on-partition layout: node rows of 1024 batch values (bf16, 2KB)
  - xT dram input [1024, 1024] bf16 (host-pretransposed x slice)
  - arena dram [2560, 1024] bf16: levels 0..4 outputs (row = node)
  - per level, per source kind (X rows from xT, OLD rows from arena levels
    <= l-2, FRESH rows from arena level l-1): one dma_gather pulls the
    dup-folded distinct source rows into stage sbuf [128, S_l, 1024];
    gathered index j lands at (partition j%128, chunk j//128)
  - per (level, set of 16 groups): PSUM [128, 512] x2 n-tiles accumulates
    chunk matmuls with scatter-expanded bf16 weights [128 src, 128 out]
  - ScalarE: tanh(psum + bias) -> bf16 newout -> arena store (fp32 for
    level 5 -> outT [512, 1024] output; host re-transposes/reorders)
"""

from contextlib import ExitStack

import ml_dtypes
import numpy as np

B = 8192
N_IN = 1024
N_LEVELS = 6
G = 64
K_IN = 32
K_OUT = 8
P = 128
SETS = 4
GPS = G // SETS  # 16 groups per set
NCORES = 8
BC = B // NCORES  # 1024 batch per core
NT = 2  # n-tiles
NTW = BC // NT  # 512

X_KIND, OLD_KIND, FRESH_KIND = 0, 1, 2
KINDS = (X_KIND, OLD_KIND, FRESH_KIND)
ARENA_ROWS = 512 * (N_LEVELS - 1)

_BF16 = ml_dtypes.bfloat16


def _node_arena_row(col):
    c = col - N_IN
    lvl = c // 512
    j = c % 512
    g = j // K_OUT
    o = j % K_OUT
    return 512 * lvl + 128 * (g // GPS) + K_OUT * (g % GPS) + o, lvl


def _wrap16(idx_list):
    """[n] int16 (n % 16 == 0) -> [128, n//16] wrapped + replicated."""
    w = idx_list.reshape(-1, 16).T  # [16, n//16]
    return np.tile(w, (8, 1)).astype(np.int16)


def _build_plan(idx, kernels, biases):
    idx = np.asarray(idx)
    kernels = np.asarray(kernels, dtype=np.float32)
    biases = np.asarray(biases, dtype=np.float32)

    # per (level, kind): flat source list (set-contiguous), gather descriptor
    gathers = {}  # (lvl, kind) -> dict(slot_lo, n_chunks, num_idxs, n_valid,
    #                                    col_lo, idx16 cols)
    mms = []  # per (lvl, set): list of (w_mat [128,128] fp32, slot)
    level_slots = []
    idx_cols = []  # list of [128, cols] int16 blocks
    col_pos = 0

    for lvl in range(N_LEVELS):
        # collect per (set, kind) distinct sources and their refs
        srcs = {(s, k): [] for s in range(SETS) for k in KINDS}
        refs = {(s, k): {} for s in range(SETS) for k in KINDS}
        for s in range(SETS):
            for gl in range(GPS):
                g = GPS * s + gl
                for i in range(K_IN):
                    col = int(idx[lvl, g, i])
                    if col < N_IN:
                        kind, src = X_KIND, col
                    else:
                        arow, src_lvl = _node_arena_row(col)
                        kind = FRESH_KIND if src_lvl == lvl - 1 else OLD_KIND
                        src = arow
                    d = refs[(s, kind)]
                    if src not in d:
                        d[src] = []
                        srcs[(s, kind)].append(src)
                    d[src].append((gl, i, g))

        slot = 0
        set_mms = [[] for _ in range(SETS)]
        for kind in KINDS:
            flat = []  # (src_row, set)
            for s in range(SETS):
                flat.extend((r, s) for r in srcs[(s, kind)])
            if not flat:
                continue
            n_valid = len(flat)
            num_idxs = (n_valid + 15) // 16 * 16
            n_chunks = (num_idxs + P - 1) // P
            rows = np.full((num_idxs,), -1, dtype=np.int16)
            for j, (r, s) in enumerate(flat):
                rows[j] = r
            idx_cols.append(_wrap16(rows))
            # per chunk, per set with rows in it: one W matrix
            for c in range(n_chunks):
                seg = flat[c * P : (c + 1) * P]
                by_set = {}
                for j, (r, s) in enumerate(seg):
                    by_set.setdefault(s, []).append((j, r))
                for s, entries in by_set.items():
                    w = np.zeros((P, P), dtype=np.float32)
                    for j, r in entries:
                        for gl, i, g in refs[(s, kind)][r]:
                            w[j, K_OUT * gl : K_OUT * (gl + 1)] += kernels[
                                lvl, g, i, :
                            ]
                    set_mms[s].append((w, slot + c))
            gathers[(lvl, kind)] = dict(
                slot_lo=slot,
                n_chunks=n_chunks,
                num_idxs=num_idxs,
                n_valid=n_valid,
                col_lo=col_pos,
                cols=num_idxs // 16,
                rows=rows,
            )
            col_pos += num_idxs // 16
            slot += n_chunks
        level_slots.append(slot)
        mms.append(set_mms)

    idx16_all = np.concatenate(idx_cols, axis=1)  # [128, col_pos]

    # pack all W matrices; assign indices
    w_list = []
    mm_sched = []  # [lvl][set] -> list of (w_idx, slot)
    for lvl in range(N_LEVELS):
        per_set = []
        for s in range(SETS):
            lst = []
            for w, slot in mms[lvl][s]:
                lst.append((len(w_list), slot))
                w_list.append(w)
            per_set.append(lst)
        mm_sched.append(per_set)
    W_all = np.stack([w.astype(_BF16) for w in w_list])  # [n_w, 128, 128]
    # host-prearranged layouts so the device-side const loads are contiguous
    # per partition: W_host[k, c*128+m] = W_all[c, k, m]
    W_host = np.ascontiguousarray(W_all.transpose(1, 0, 2).reshape(P, -1))

    bias_all = np.zeros((N_LEVELS, SETS, P), dtype=np.float32)
    for lvl in range(N_LEVELS):
        for s in range(SETS):
            for gl in range(GPS):
                g = GPS * s + gl
                bias_all[lvl, s, K_OUT * gl : K_OUT * (gl + 1)] = biases[lvl, g]

    bias_host = np.ascontiguousarray(
        bias_all.transpose(2, 0, 1).reshape(P, N_LEVELS * SETS)
    )
    return dict(
        gathers=gathers,
        mm_sched=mm_sched,
        level_slots=level_slots,
        W_all=W_all,
        W_host=W_host,
        idx16_all=idx16_all,
        bias_all=bias_all,
        bias_host=bias_host,
    )


def _out_row_to_col():
    rows = np.zeros(512, dtype=np.int64)
    for r in range(512):
        s, within = divmod(r, 128)
        gl, o = divmod(within, K_OUT)
        rows[r] = (GPS * s + gl) * K_OUT + o
    return rows


def _build_bass(plan, max_levels=N_LEVELS):
    import concourse.mybir as mybir
    import concourse.tile as tile
    from concourse import bacc

    gathers = plan["gathers"]
    mm_sched = plan["mm_sched"]
    n_w = plan["W_all"].shape[0]
    n_cols = plan["idx16_all"].shape[1]
    S_max = max(plan["level_slots"])

    nc = bacc.Bacc(
        "TRN2", target_bir_lowering=False, debug=False, num_devices=NCORES
    )
    xT = nc.dram_tensor("xT", [N_IN, BC], mybir.dt.bfloat16, kind="ExternalInput")
    W_in = nc.dram_tensor(
        "W_host", [P, n_w * P], mybir.dt.bfloat16, kind="ExternalInput"
    )
    idx_in = nc.dram_tensor(
        "idx16_all", [P, n_cols], mybir.dt.int16, kind="ExternalInput"
    )
    bias_in = nc.dram_tensor(
        "bias_host", [P, N_LEVELS * SETS], mybir.dt.float32, kind="ExternalInput"
    )
    outT = nc.dram_tensor("outT", [512, BC], mybir.dt.float32, kind="ExternalOutput")
    arena = nc.dram_tensor("arena", [ARENA_ROWS, BC], mybir.dt.bfloat16)

    with tile.TileContext(nc) as tc, ExitStack() as ctx:
        const_pool = ctx.enter_context(tc.tile_pool(name="const", bufs=1))
        stage_pool = ctx.enter_context(tc.tile_pool(name="stage", bufs=1))
        psum_pool = ctx.enter_context(
            tc.tile_pool(name="psum", bufs=8, space="PSUM")
        )
        newout_pool = ctx.enter_context(tc.tile_pool(name="newout", bufs=2))
        newout5_pool = ctx.enter_context(tc.tile_pool(name="newout5", bufs=1))

        w_t = const_pool.tile([P, n_w, P], mybir.dt.bfloat16)
        nc.sync.dma_start(w_t[:], W_in.ap().rearrange("k (c m) -> k c m", m=P))
        idx_t = const_pool.tile([P, n_cols], mybir.dt.int16)
        nc.sync.dma_start(idx_t[:], idx_in.ap())
        bias_t = const_pool.tile([P, N_LEVELS, SETS], mybir.dt.float32)
        nc.sync.dma_start(
            bias_t[:], bias_in.ap().rearrange("k (l s) -> k l s", s=SETS)
        )

        stages = []
        for i in range(2):
            st = stage_pool.tile([P, S_max, BC], mybir.dt.bfloat16, tag=f"st{i}")
            nc.gpsimd.memset(st[:], 0.0)
            stages.append(st)

        # A single dma_gather's descriptors must fit in the SWDGE ring
        # (dynamic_dma_scratch_size/16 = 1024 descs) or the Q7 waits forever;
        # split into <=512-index sub-gathers, aligned to chunk boundaries.
        GMAX = 512

        def emit_gather(lvl, kind):
            gd = gathers.get((lvl, kind))
            if gd is None:
                return
            st = stages[lvl % 2]
            lo = gd["slot_lo"]
            src = xT if kind == X_KIND else arena
            rows = gd["rows"]
            for start in range(0, gd["num_idxs"], GMAX):
                n_sub = min(GMAX, gd["num_idxs"] - start)
                sub_rows = rows[start : start + n_sub]
                n_valid = int((sub_rows >= 0).sum())
                nc.gpsimd.dma_gather(
                    out_ap=st[
                        :,
                        lo + start // P : lo + start // P + (n_sub + P - 1) // P,
                        :,
                    ],
                    in_ap=src.ap(),
                    idxs_ap=idx_t[
                        :,
                        gd["col_lo"] + start // 16 : gd["col_lo"]
                        + (start + n_sub) // 16,
                    ],
                    num_idxs=n_sub,
                    num_idxs_reg=n_valid,
                    elem_size=BC,
                )

        emit_gather(0, X_KIND)
        if max_levels > 1:
            emit_gather(1, X_KIND)

        for lvl in range(max_levels):
            st = stages[lvl % 2]
            if lvl >= 1:
                emit_gather(lvl, FRESH_KIND)
            if lvl < max_levels - 1:
                new_t = newout_pool.tile(
                    [P, SETS, BC], mybir.dt.bfloat16, tag="newout"
                )
            else:
                new_t = newout5_pool.tile(
                    [P, SETS, BC], mybir.dt.float32, tag="newout5"
                )
            for s in range(SETS):
                sched = mm_sched[lvl][s]
                assert sched
                for n in range(NT):
                    acc = psum_pool.tile([P, NTW], mybir.dt.float32)
                    for ci, (wi, slot) in enumerate(sched):
                        nc.tensor.matmul(
                            acc[:],
                            w_t[:, wi, :],
                            st[:, slot, n * NTW : (n + 1) * NTW],
                            start=(ci == 0),
                            stop=(ci == len(sched) - 1),
                        )
                    nc.scalar.activation(
                        new_t[:, s, n * NTW : (n + 1) * NTW],
                        acc[:],
                        mybir.ActivationFunctionType.Tanh,
                        bias=bias_t[:, lvl, s : s + 1],
                    )
            if lvl < max_levels - 1:
                nc.sync.dma_start(
                    arena.ap()[512 * lvl : 512 * (lvl + 1), :].rearrange(
                        "(s p) b -> p s b", p=P
                    ),
                    new_t[:],
                )
            else:
                nc.sync.dma_start(
                    outT.ap().rearrange("(s p) b -> p s b", p=P), new_t[:]
                )
            if lvl + 2 < max_levels:
                emit_gather(lvl + 2, X_KIND)
                emit_gather(lvl + 2, OLD_KIND)

    nc.compile()
    return nc


_CACHE = {}


def _get_nc(plan):
    if "nc" not in _CACHE:
        _CACHE["nc"] = _build_bass(plan)
    return _CACHE["nc"]


def _run(x, kernels, biases, idx, trace=False):
    from concourse.bass_utils import run_bass_kernel_spmd

    x = np.asarray(x)
    plan = _build_plan(idx, kernels, biases)
    nc = _get_nc(plan)

    in_maps = []
    for c in range(NCORES):
        xT_c = np.ascontiguousarray(x[c * BC : (c + 1) * BC, :].T).astype(_BF16)
        in_maps.append(
            {
                "xT": xT_c,
                "W_host": plan["W_host"],
                "idx16_all": plan["idx16_all"],
                "bias_host": plan["bias_host"],
            }
        )
    res = run_bass_kernel_spmd(
        nc, in_maps, core_ids=list(range(NCORES)), trace=trace
    )
    r2c = _out_row_to_col()
    out = np.zeros((B, G * K_OUT), dtype=np.float32)
    for c in range(NCORES):
        oT = np.asarray(res.results[c]["outT"])  # [512, BC]
        out[c * BC : (c + 1) * BC, r2c] = oT.T
    return out, res


def kernel(x, kernels, biases, idx):
    out, _ = _run(x, kernels, biases, idx, trace=False)
    return out
